# revision 5
# baseline (speedup 1.0000x reference)
"""Cumulative VWAP kernel for Trainium2 (Bass/Tile), data-parallel over 8 cores.

vwap[:, t] = cumsum(s*v)[:, t] / (cumsum(v)[:, t] + 1e-8),  vwap[:, 0] = s[:, 0]

Sharding: num_paths (axis 0) split evenly across 8 NeuronCores; the cumsum
runs along the time axis, which stays local to each core (no collectives).

The problem is memory-bound (96 MiB/core of HBM traffic at f32).  The rel-err
budget (2e-2) is spent on lower-precision I/O, halving the traffic:
  * inputs are host-converted to fp16 (volume pre-scaled by 2^-7 so it fits
    fp16 range; the scale cancels exactly in the VWAP ratio),
  * the output is stored bf16 and host-upcast to f32.
  -> 48 MiB/core.  Measured end-to-end rel err ~1.2e-2.

Both cumsums run as hand-written custom-DVE uop programs in the 2X_1PORT
perf mode (2 fp16 elements/cycle): the engine reads a packed fp16 pair per
32-bit port read, computes  p = x_e + x_o;  z += p;  y_e = z - x_o; y_o = z
(with the s*v multiply fused in the feed-forward stage for the pv scan),
and writes a packed bf16 pair per 32-bit write (WR0_LO/WR0_HI), with the
running sum z in a mid-pipeline CURR_ALU_OUT flop -- 2x the stock scan
throughput.  The 1x table slot keeps an auto-lowered fallback program with
identical semantics (the RTL silently falls back if the mem-pattern
disqualifies).  perf_max=1 (byte36[7:6]) maps to PerfModeType::TwoSrc, so
only the 2X_1PORT slot is engine-reachable; both ops declare rd1_en=1 for
exactly this reason (OneSrc would expose the unimplemented 2-port modes).

Per-core dataflow, per [128, 4096] tile (16 tiles per core):
  DMA  : load stock fp16, volume fp16 (1 MiB each, contiguous, nc.sync)
  ACT  : save col 0 of stock (t==0 fix is an exact copy of s0)
  DVE  : pv = PV2X(s, v)       bf16 out, ~1.7 us
  DVE  : vc = V2X(v, s)        bf16 out, ~2.9 us (s drained, ignored)
  ACT  : ln = Ln(vc)           f32 (ln must stay f32: abs err -> rel err)
  ACT  : r  = Exp(-ln)         bf16 (reciprocal via exp(-ln); ACT Reciprocal
                               is banned for accuracy, and the single
                               combined Ln+Exp table set avoids the ~2.7us
                               per-tile table reloads)
  DVE  : vwap = pv * r         bf16 tensor_tensor in 2x mode, two halves
  ACT  : restore col 0
  DMA  : store vwap halves (nc.scalar queue, issued mid-tile)
Engine busy per core: DVE ~105us, ACT ~115us, DMA ~145us -> DMA-bound.
Measured ~133us/rep (repeat-slope, 8 cores): 2.2x over the f32 baseline.
"""

import numpy as np

NUM_PATHS = 16384
TIME = 4096
N_CORES = 8
ROWS = NUM_PATHS // N_CORES  # rows per core
P = 128  # SBUF partitions

_CACHE = {}

_COMBINED_SET = "natural_log_exp_and_others"


def _single_act_set_bacc():
    import concourse.bacc as bacc

    class SingleActSetBacc(bacc.Bacc):
        """Restrict the activation-table-load pass to one set holding
        Ln+Exp+Copy so alternating Ln/Exp doesn't reload tables every tile."""

        def insert_act_table_loads(self):
            import bass_rust
            import concourse.mybir as mybir
            from concourse.hw_specs import get_activation_tables

            has_activation = any(
                isinstance(i, mybir.InstActivation)
                for b in self.main_func.blocks
                for i in b.instructions
            )
            if not has_activation:
                return
            tables = [
                (name, fns if name == _COMBINED_SET else set())
                for name, fns in get_activation_tables(self.m.arch).items()
            ]
            bass_rust.insert_act_table_loads(self, tables)

    return SingleActSetBacc


# --------------------------------------------------------------------------
# Hand-written 2X_1PORT pair-scan uop programs.
# --------------------------------------------------------------------------

def _mk_pv2x_uops():
    """cumsum(in0*in1), one packed fp16 pair/cycle:
    m_e = s_e*v_e; m_o = s_o*v_o; p = m_e+m_o; z += p; y_e = z-m_o; y_o = z.
    Output packing (WR0_LO = even via ALU lane, WR0_HI = odd via delay 0)
    follows the stock tensor_tensor 2x_1p program."""
    from concourse.dve_uop import (
        ENABLE, AluInp, AluOp, DelayInp, InpSel, OutPath, OutSel,
        Trigger, UopConfig,
    )

    def base_inputs(u):
        u.enable_input(InpSel.SRC_0, 0)        # s_e -> stage0 ALU A
        u.enable_input(InpSel.SRC_1, 1)        # v_e -> PREV_DELAY_0
        u.enable_input(InpSel.SRC_0_HI, 2)     # s_o -> PREV_DELAY_1
        u.enable_input(InpSel.SRC_1_HI, 3)     # v_o -> PREV_DELAY_2
        u.enable_input(InpSel.ZERO, 4)         # 0   -> PREV_DELAY_3

    # seed uop: one dummy element seeds stage3's CURR_ALU_OUT (z) with 0
    seed = UopConfig()
    base_inputs(seed)
    seed.repeat_count = 1
    seed.trigger = (Trigger.COUNT, Trigger.NONE, Trigger.NONE)
    seed.next_uop = (1, 0, 0)
    d = seed.datapath_config
    for k in range(3):
        d[k].pass_through_alu()
        d[k].pass_through_delay(3)             # carry ZERO to stage 3
    d[3].enable_alu(AluOp.BYPASS, AluInp.PREV_DELAY_3)   # z flop <- 0
    for k in range(4, 8):
        d[k].pass_through_alu()

    # steady uop: one packed pair per cycle
    st = UopConfig()
    base_inputs(st)
    st.require_inp0 = ENABLE
    st.require_inp1 = ENABLE
    st.trigger = (Trigger.SRC_TENSOR_DONE, Trigger.NONE, Trigger.NONE)
    st.next_uop = (0, 0, 0)
    st.enable_output(OutSel.ALU_OUT, OutPath.WR0_LO)     # y_even
    st.enable_output(OutSel.DELAY_0, OutPath.WR0_HI)     # y_odd = z
    d = st.datapath_config
    d[0].enable_alu(AluOp.MULTIPLY, AluInp.PREV_ALU_OUT, AluInp.PREV_DELAY_0)
    d[0].pass_through_delay(1, 2)              # s_o, v_o
    d[1].enable_alu(AluOp.MULTIPLY, AluInp.PREV_DELAY_1, AluInp.PREV_DELAY_2)
    d[1].enable_delay_from_src(DelayInp.PREV_ALU_OUT, 0)  # capture m_e
    d[2].enable_alu(AluOp.ADD, AluInp.PREV_ALU_OUT, AluInp.PREV_DELAY_0)  # p
    d[2].enable_delay_from_src(DelayInp.PREV_ALU_OUT, 1)  # capture m_o
    d[3].enable_alu(AluOp.ADD, AluInp.CURR_ALU_OUT, AluInp.PREV_ALU_OUT)  # z
    d[3].pass_through_delay(1)                 # m_o
    d[4].enable_alu(AluOp.SUBTRACT, AluInp.PREV_ALU_OUT, AluInp.PREV_DELAY_1)
    d[4].enable_delay_from_src(DelayInp.PREV_ALU_OUT, 0)  # capture z
    for k in range(5, 8):
        d[k].pass_through_alu()
        d[k].pass_through_delay(0)
    return [seed, st]


def _mk_v2x_uops():
    """cumsum(in0) pair-scan; SRC_1 is required and drained (keeps the op in
    the TwoSrc perf class, where only 2X_1PORT is reachable) but ignored."""
    from concourse.dve_uop import (
        ENABLE, AluInp, AluOp, DelayInp, InpSel, OutPath, OutSel,
        Trigger, UopConfig,
    )

    def base_inputs(u):
        u.enable_input(InpSel.SRC_0, 0)        # x_e -> stage0 ALU A
        u.enable_input(InpSel.SRC_1, 1)        # drained, value ignored
        u.enable_input(InpSel.SRC_0_HI, 2)     # x_o -> PREV_DELAY_1
        u.enable_input(InpSel.SRC_0_HI, 3)     # x_o -> PREV_DELAY_2
        u.enable_input(InpSel.ZERO, 4)         # 0   -> PREV_DELAY_3

    seed = UopConfig()
    base_inputs(seed)
    seed.repeat_count = 1
    seed.trigger = (Trigger.COUNT, Trigger.NONE, Trigger.NONE)
    seed.next_uop = (1, 0, 0)
    d = seed.datapath_config
    d[0].pass_through_alu()
    d[0].pass_through_delay(3)
    d[1].enable_alu(AluOp.BYPASS, AluInp.PREV_DELAY_3)   # z flop <- 0
    for k in range(2, 8):
        d[k].pass_through_alu()

    st = UopConfig()
    base_inputs(st)
    st.require_inp0 = ENABLE
    st.require_inp1 = ENABLE
    st.trigger = (Trigger.SRC_TENSOR_DONE, Trigger.NONE, Trigger.NONE)
    st.next_uop = (0, 0, 0)
    st.enable_output(OutSel.ALU_OUT, OutPath.WR0_LO)     # y_even
    st.enable_output(OutSel.DELAY_0, OutPath.WR0_HI)     # y_odd = z
    d = st.datapath_config
    d[0].enable_alu(AluOp.ADD, AluInp.PREV_ALU_OUT, AluInp.PREV_DELAY_1)  # p
    d[0].pass_through_delay(2)
    d[1].enable_alu(AluOp.ADD, AluInp.CURR_ALU_OUT, AluInp.PREV_ALU_OUT)  # z
    d[1].pass_through_delay(2)
    d[2].enable_alu(AluOp.SUBTRACT, AluInp.PREV_ALU_OUT, AluInp.PREV_DELAY_2)
    d[2].enable_delay_from_src(DelayInp.PREV_ALU_OUT, 0)  # capture z
    for k in range(3, 8):
        d[k].pass_through_alu()
        d[k].pass_through_delay(0)
    return [seed, st]


def _register_pair_ops():
    """Register the pair-scan DveOps; the hand 2x program is injected via
    the compile cache so DveOp.compile returns it table-generation-time."""
    import concourse.dve_ops as dve_ops
    from concourse.dve_ops import DveOp, _COMPILE_CACHE
    from concourse.dve_spec import AluOp, Spec, Src0, Src1, lower, scan
    from concourse.dve_uop import DveOpSpec

    def mk(name, spec, uops_2x):
        for o in dve_ops.OPS:
            if o.name == name:
                return o
        op = DveOp(name, spec, subdim=False, uops_sha={})
        dve_ops.OPS.append(op)
        dve_ops.CUSTOM_DVE_SPECS[name] = spec
        dve_ops._SUB_OPCODE_FOR_NAME[name] = (
            dve_ops._CUSTOM_DVE_ROW_BASE + len(dve_ops.OPS) - 1
        )
        assert dve_ops._SUB_OPCODE_FOR_NAME[name] < 0x20
        for ver in ("v3", "v4"):
            s = DveOpSpec(
                name=name,
                opcode=dve_ops.get_dve_sub_opcode(name),
                uops=lower(spec, ver=ver),
                uops_2x=uops_2x,
                perf_max=1,
                rd1_en=True,
            )
            op.uops_sha[ver] = s.sha(ver)
            _COMPILE_CACHE[(name, ver)] = s
        return op

    pv = mk(
        "PV2X_ANT",
        Spec(
            body=scan(AluOp.ADD, Src0 * Src1),
            reference=lambda in0, in1, s0, s1, imm2: np.cumsum(
                in0.astype(np.float32) * in1.astype(np.float32),
                axis=-1, dtype=np.float32,
            ),
        ),
        _mk_pv2x_uops(),
    )
    v = mk(
        "V2X_ANT",
        Spec(
            body=scan(AluOp.ADD, Src0),
            reference=lambda in0, in1, s0, s1, imm2: np.cumsum(
                in0.astype(np.float32), axis=-1, dtype=np.float32
            ),
        ),
        _mk_v2x_uops(),
    )
    return pv, v


def _build(rows=ROWS, time=TIME, bufs=3, reps=1, store_splits=1,
           load_split=False, store_alt=False):
    import concourse.tile as tile
    import concourse.mybir as mybir

    pv_op, v_op = _register_pair_ops()
    nc = _single_act_set_bacc()("TRN2", target_bir_lowering=False, debug=False)
    f32 = mybir.dt.float32
    f16 = mybir.dt.float16
    bf16 = mybir.dt.bfloat16
    stock = nc.dram_tensor("stock_paths", [rows, time], f16, kind="ExternalInput").ap()
    vol = nc.dram_tensor("volume_paths", [rows, time], f16, kind="ExternalInput").ap()
    out = nc.dram_tensor("vwap_out", [rows, time], bf16, kind="ExternalOutput").ap()

    Ln = mybir.ActivationFunctionType.Ln
    Exp = mybir.ActivationFunctionType.Exp

    n_tiles = rows // P
    with tile.TileContext(nc) as tc:
        with (
            tc.tile_pool(name="big", bufs=bufs) as big,
            tc.tile_pool(name="small", bufs=bufs) as small,
        ):
            for i in range(n_tiles * reps):
                r0 = (i % n_tiles) * P
                ts = big.tile([P, time], f16, tag="ts")
                tv = big.tile([P, time], f16, tag="tv")
                to = big.tile([P, time], bf16, tag="to")
                pv = big.tile([P, time], bf16, tag="pv")
                vc = big.tile([P, time], bf16, tag="vc")
                ln = big.tile([P, time], f32, tag="ln")
                t0 = small.tile([P, 1], f32, tag="t0")
                nc.sync.dma_start(ts[:], stock[r0 : r0 + P, :])
                vol_eng = nc.scalar if load_split else nc.sync
                vol_eng.dma_start(tv[:], vol[r0 : r0 + P, :])
                nc.scalar.copy(t0[:], ts[:, 0:1])
                i1 = nc.vector._custom_dve(pv_op, out=pv[:], in0=ts[:], in1=tv[:])
                i2 = nc.vector._custom_dve(v_op, out=vc[:], in0=tv[:], in1=ts[:])
                i1.ins.perf_max = 1
                i2.ins.perf_max = 1
                nc.scalar.activation(ln[:], vc[:], Ln)
                nc.scalar.activation(vc[:], ln[:], Exp, scale=-1.0)
                w = time // store_splits
                for k in range(store_splits):
                    sl = slice(k * w, (k + 1) * w)
                    nc.vector.tensor_mul(to[:, sl], pv[:, sl], vc[:, sl])
                    if k == 0:
                        nc.scalar.copy(to[:, 0:1], t0[:])
                    st = nc.sync if (store_alt and k % 2 == 1) else nc.scalar
                    st.dma_start(out[r0 : r0 + P, sl], to[:, sl])
    nc.compile()
    return nc


def _get_nc():
    if "nc" not in _CACHE:
        _CACHE["nc"] = _build()
    return _CACHE["nc"]


def _prep_inputs(stock_paths, volume_paths):
    s16 = stock_paths.astype(np.float16)
    v16 = (volume_paths * np.float32(2.0 ** -7)).astype(np.float16)
    return s16, v16


def kernel(stock_paths: np.ndarray, volume_paths: np.ndarray) -> np.ndarray:
    from concourse.bass_utils import run_bass_kernel_spmd

    stock_paths = np.ascontiguousarray(stock_paths, dtype=np.float32)
    volume_paths = np.ascontiguousarray(volume_paths, dtype=np.float32)
    assert stock_paths.shape == (NUM_PATHS, TIME)

    s16, v16 = _prep_inputs(stock_paths, volume_paths)
    nc = _get_nc()
    in_maps = [
        {
            "stock_paths": s16[i * ROWS : (i + 1) * ROWS],
            "volume_paths": v16[i * ROWS : (i + 1) * ROWS],
        }
        for i in range(N_CORES)
    ]
    res = run_bass_kernel_spmd(nc, in_maps, core_ids=list(range(N_CORES)))
    return np.concatenate(
        [r["vwap_out"].astype(np.float32) for r in res.results], axis=0
    )
